# revision 1
# baseline (speedup 1.0000x reference)
"""MLA prefill kernel for Trainium2, 8 NeuronCores.

Sharding: data-parallel over batch (2) x tensor-parallel over heads
(16 heads -> 4 per core).  Core c handles batch c//4, head group c%4.
Each core computes its full attention block plus a partial output
projection; the host sums the 4 per-group partials per batch.

Layout strategy: everything is computed transposed ([feature, L]) so
matmul lhsT/rhs operands are produced directly, except V (L-major for
the PV matmul), which is spilled to a DRAM scratch and re-streamed.
Scores are computed transposed (S^T = K Q^T, [Lk, Lq]) so softmax's
sum runs through the PV matmul via an appended ones-column; exp needs
no max-subtraction (scores are O(10), fp32 exp is safe).  RoPE pair
mixing runs along partitions, done with a +-1 pair-swap matmul (J) on
the tensor engine plus two elementwise multiplies and an add.

Matmuls use float32r (1 cycle/row at N>=512, ~TF32 accuracy).
"""

import math
import sys

sys.path.insert(0, "/opt/trn_rl_repo")

import numpy as np

import concourse.bass as bass
import concourse.mybir as mybir
import concourse.tile as tile
from concourse.bass import ds
from concourse.bass_utils import run_bass_kernel_spmd

H, DH, RK, RD = 16, 128, 512, 64
B, L, E = 2, 2048, 2048
HPG = 4                      # heads per core
NCORE = 8
DV = DH + RD                 # 192
SCALE = 1.0 / math.sqrt(DV)
CH = 512                     # Lq chunk
NCH = L // CH                # 4
LT = L // 128                # 16 key tiles
ET = E // 128                # 16
W1C = HPG * DH + RK + HPG * RD + RD   # 1344 fused QKV columns
VROW = HPG * (DV + 1)        # 772: per-head 192 v dims + ones col

F32 = mybir.dt.float32
F32R = mybir.dt.float32r
AF = mybir.ActivationFunctionType

_CACHE = {}


def _split_excess_waits(nc, limit=1):
    """walrus on this toolchain accepts at most one sem-wait per
    instruction; hoist extras onto same-engine no-ops just before."""
    f = nc.m.functions[0]
    for bb in f.blocks:
        new_list = []
        changed = False
        for inst in bb.instructions:
            si = inst.sync_info
            if si is not None and si.on_wait is not None and len(si.on_wait) > limit:
                waits = list(si.on_wait)
                changed = True
                n = 0
                while len(waits) > limit:
                    chunk, waits = waits[:limit], waits[limit:]
                    new_list.append(mybir.InstNoOp(
                        name=f"{inst.name}-ws{n}",
                        sync_info=mybir.SyncInfo(on_wait=chunk, on_update=[]),
                        bass_nofuse=True,
                        engine=inst.engine,
                    ))
                    n += 1
                inst.sync_info = mybir.SyncInfo(on_wait=waits, on_update=si.on_update)
            new_list.append(inst)
        if changed:
            bb.instructions[:] = new_list
    return nc


def _build():
    nc = bass.Bass(target_bir_lowering=False, trn_type="TRN2")

    xt = nc.dram_tensor("xt", [NCH, 128, ET, CH], F32R, kind="ExternalInput")
    w1 = nc.dram_tensor("w1", [11, 128, ET, 128], F32R, kind="ExternalInput")
    wuk = nc.dram_tensor("wuk", [RK, HPG * DH], F32R, kind="ExternalInput")
    wuv = nc.dram_tensor("wuv", [RK, HPG * DV], F32R, kind="ExternalInput")
    woa = nc.dram_tensor("woa", [E // CH, 128, HPG, CH], F32R, kind="ExternalInput")
    wob = nc.dram_tensor("wob", [E // CH, RD, HPG, CH], F32R, kind="ExternalInput")
    cost = nc.dram_tensor("cost", [128, L], F32R, kind="ExternalInput")
    sint = nc.dram_tensor("sint", [128, L], F32R, kind="ExternalInput")
    jt = nc.dram_tensor("jt", [128, 128], F32R, kind="ExternalInput")
    triu = nc.dram_tensor("triu", [128, 128], F32R, kind="ExternalInput")
    onesc = nc.dram_tensor("onesc", [128, HPG], F32R, kind="ExternalInput")
    outt = nc.dram_tensor("outt", [L, E], F32, kind="ExternalOutput")

    from contextlib import ExitStack

    with tile.TileContext(nc) as tc:
        with ExitStack() as ctx:
            pool_specs = [
                ("consts", 1, None), ("res", 1, None), ("dscr", 1, "DRAM"),
                ("rrd_p", 2, "DRAM"), ("xt_p", 1, None), ("w1_p", 1, None),
                ("qt_p", 2, None), ("rq_p", 2, None), ("ckv_p", 1, None),
                ("vst_p", 4, None), ("vsr_p", 2, None), ("p_p", 3, None),
                ("tmp_p", 1, None), ("rb_p", 1, None), ("ot_p", 1, None),
                ("wo_p", 1, None), ("fin_p", 2, None), ("ps_p", 8, "PSUM"),
            ]
            pools = {}
            for pname, pbufs, pspace in pool_specs:
                kw = {"name": pname, "bufs": pbufs}
                if pspace:
                    kw["space"] = pspace
                pools[pname] = ctx.enter_context(tc.tile_pool(**kw))
            (consts, res, dscr, rrd_p, xt_p, w1_p, qt_p, rq_p, ckv_p, vst_p,
             vsr_p, p_p, tmp_p, rb_p, ot_p, wo_p, fin_p, ps_p) = (
                pools[s[0]] for s in pool_specs)
            def psum():
                return ps_p.tile([128, 512], F32, tag="ps", name="ps")

            # ---- constants / resident weights
            jt_t = consts.tile([128, 128], F32R, tag="jt", name="jt")
            nc.sync.dma_start(out=jt_t[:], in_=jt.ap())
            tri_t = consts.tile([128, 128], F32R, tag="tri", name="tri")
            nc.sync.dma_start(out=tri_t[:], in_=triu.ap())
            wukt = res.tile([128, RK // 128, HPG * DH], F32R, tag="wukt", name="wukt")
            nc.sync.dma_start(out=wukt[:], in_=wuk.ap().rearrange("(t p) n -> p t n", p=128))
            wuvt = res.tile([128, RK // 128, HPG * DV], F32R, tag="wuvt", name="wuvt")
            nc.sync.dma_start(out=wuvt[:], in_=wuv.ap().rearrange("(t p) n -> p t n", p=128))

            ktc = res.tile([128, HPG, L], F32R, tag="ktc", name="ktc")     # K content, transposed
            rkd = res.tile([128, L], F32R, tag="rkd", name="rkd")          # roped k_rope, duplicated rows
            vd = dscr.tile([LT, 128, VROW], F32R, tag="vd", name="vd")    # V spill (L-major + ones)

            # d-tiles of the fused QKV projection: (offset, width, kind, idx)
            dtiles = (
                [(128 * i, 128, "q", i) for i in range(HPG)]
                + [(HPG * DH + 128 * i, 128, "ckv", i) for i in range(RK // 128)]
                + [(HPG * DH + RK + 128 * i, 128, "rq", i) for i in range(2)]
                + [(HPG * DH + RK + HPG * RD, RD, "rk", 0)]
            )

            for c in range(NCH):
                ccols = ds(c * CH, CH)

                # ================= QKV(c): [1344, CH] = W1^T @ x^T =======
                xtt = xt_p.tile([128, ET, CH], F32R, tag="xtt", name="xtt")
                nc.sync.dma_start(out=xtt[:], in_=xt.ap()[c])
                cos_t = rb_p.tile([128, CH], F32R, tag="cosc", name="cosc")
                nc.sync.dma_start(out=cos_t[:], in_=cost.ap()[:, ccols])
                sin_t = rb_p.tile([128, CH], F32R, tag="sinc", name="sinc")
                nc.sync.dma_start(out=sin_t[:], in_=sint.ap()[:, ccols])
                qtc = qt_p.tile([128, HPG, CH], F32R, tag="qtc", name="qtc")
                rq = rq_p.tile([128, 2, CH], F32R, tag="rq", name="rq")
                ckv = ckv_p.tile([128, RK // 128, CH], F32R, tag="ckv", name="ckv")

                for di, (doff, dw, kind, idx) in enumerate(dtiles):
                    w1s = w1_p.tile([128, ET, 128], F32R, tag="w1s", name="w1s")
                    nc.sync.dma_start(out=w1s[:, :, :dw], in_=w1.ap()[di, :, :, :dw])
                    ps = psum()
                    for e in range(ET):
                        nc.tensor.matmul(ps[:dw, :CH], w1s[:, e, :dw], xtt[:, e, :],
                                         start=(e == 0), stop=(e == ET - 1))
                    if kind == "q":
                        nc.scalar.copy(out=qtc[:, idx, :], in_=ps[:, :CH])
                    elif kind == "ckv":
                        nc.vector.tensor_copy(ckv[:, idx, :], ps[:, :CH])
                    elif kind == "rq":
                        nc.vector.tensor_copy(rq[:, idx, :], ps[:, :CH])
                    else:  # pre-rope k_rope at partitions 0:64
                        nc.vector.tensor_copy(rkd[0:RD, ccols], ps[:RD, :CH])

                # ================= RoPE(c) ===============================
                # roped = R * cos + (J @ R) * sin   (pairs along partitions)
                for i in range(2):  # q_rope, two head-pair tiles
                    swp = psum()
                    nc.tensor.matmul(swp[:, :CH], jt_t[:, :], rq[:, i, :],
                                     start=True, stop=True)
                    t1 = tmp_p.tile([128, CH], F32R, tag="ropet", name="ropet")
                    nc.vector.tensor_mul(t1[:], rq[:, i, :], cos_t[:])
                    nc.vector.tensor_mul(rq[:, i, :], swp[:, :CH], sin_t[:])
                    nc.vector.tensor_add(rq[:, i, :], rq[:, i, :], t1[:])
                swp = psum()
                nc.tensor.matmul(swp[:RD, :CH], jt_t[:RD, :RD], rkd[0:RD, ccols],
                                 start=True, stop=True)
                t1 = tmp_p.tile([128, CH], F32R, tag="ropet", name="ropet")
                nc.vector.tensor_mul(t1[:RD, :], rkd[0:RD, ccols], cos_t[0:RD, :])
                nc.vector.tensor_mul(rkd[0:RD, ccols], swp[:RD, :CH], sin_t[0:RD, :])
                nc.vector.tensor_add(rkd[0:RD, ccols], rkd[0:RD, ccols], t1[:RD, :])
                # duplicate roped k_rope to partitions 64:128 (for odd heads)
                nc.sync.dma_start(out=rkd[RD:128, ccols], in_=rkd[0:RD, ccols])

                # ================= UP-K(c): K^T = Wuk^T @ c_kv^T =========
                for h in range(HPG):
                    ps = psum()
                    for kt in range(RK // 128):
                        nc.tensor.matmul(ps[:, :CH], wukt[:, kt, ds(128 * h, 128)],
                                         ckv[:, kt, :],
                                         start=(kt == 0), stop=(kt == RK // 128 - 1))
                    nc.scalar.copy(out=ktc[:, h, ccols], in_=ps[:, :CH])

                # ================= UP-V(c): V = c_kv @ Wuv (L-major) =====
                chunk_vst = []
                for lti in range(4):
                    lt = 4 * c + lti
                    vst = vst_p.tile([128, VROW], F32R, tag="vst", name="vst")
                    chunk_vst.append(vst)
                    for nb in range(2):
                        psv = psum()
                        for kt in range(RK // 128):
                            nc.tensor.matmul(psv[:, :384],
                                             ckv[:, kt, ds(128 * lti, 128)],
                                             wuvt[:, kt, ds(384 * nb, 384)],
                                             start=(kt == 0), stop=(kt == RK // 128 - 1))
                        for q in range(2):
                            hh = 2 * nb + q
                            nc.scalar.copy(out=vst[:, ds((DV + 1) * hh, DV)],
                                           in_=psv[:, ds(DV * q, DV)])
                    ones_view = vst[:].rearrange("p (h x) -> p h x", x=DV + 1)
                    nc.sync.dma_start(out=ones_view[:, :, DV], in_=onesc.ap())
                    nc.sync.dma_start(out=vd[lt], in_=vst[:])

                # ================= ATT(c): head pairs ====================
                ntk = 4 * c + 4
                for hp in range(2):
                    pvs = []
                    for q in range(2):
                        pvs.append((psum(), psum()))  # (ps1, ps2) per head
                    for t in range(ntk):
                        j = t - 4 * c
                        off = 128 * j if j >= 0 else 0
                        n = CH - off
                        if j >= 0:
                            vsr = chunk_vst[j]
                        else:
                            vsr = vsr_p.tile([128, VROW], F32R, tag="vsr", name="vsr")
                            nc.gpsimd.dma_start(out=vsr[:], in_=vd[t])
                        for q in range(2):
                            h = 2 * hp + q
                            ps1, ps2 = pvs[q]
                            hb = RD * (h % 2)
                            sps = psum()
                            nc.tensor.matmul(sps[:, ds(off, n)],
                                             ktc[:, h, ds(128 * t, 128)],
                                             qtc[:, h, ds(off, n)],
                                             start=True, stop=False)
                            nc.tensor.matmul(sps[:, ds(off, n)],
                                             rkd[hb:hb + RD, ds(128 * t, 128)],
                                             rq[hb:hb + RD, h // 2, ds(off, n)],
                                             start=False, stop=True)
                            pt = p_p.tile([128, CH], F32R, tag="pt", name="pt")
                            nc.scalar.activation(pt[:, ds(off, n)], sps[:, ds(off, n)],
                                                 AF.Exp, scale=SCALE)
                            if j >= 0:
                                nc.vector.tensor_mul(pt[:, ds(off, 128)],
                                                     pt[:, ds(off, 128)], tri_t[:])
                            nc.tensor.matmul(ps1[:, ds(off, n)],
                                             vsr[:, ds((DV + 1) * h, 128)],
                                             pt[:, ds(off, n)],
                                             start=(t == 0), stop=(t == ntk - 1),
                                             skip_group_check=True)
                            nc.tensor.matmul(ps2[:DV - DH + 1, ds(off, n)],
                                             vsr[:, ds((DV + 1) * h + DH, DV - DH + 1)],
                                             pt[:, ds(off, n)],
                                             start=(t == 0), stop=(t == ntk - 1),
                                             skip_group_check=True)
                    if hp == 0:
                        ota = ot_p.tile([128, HPG, CH], F32R, tag="ota", name="ota")
                        otb = ot_p.tile([RD, HPG, CH], F32R, tag="otb", name="otb")
                    for q in range(2):
                        h = 2 * hp + q
                        ps1, ps2 = pvs[q]
                        rr = tmp_p.tile([128, CH], F32R, tag="rr", name="rr")
                        with nc.allow_low_precision(reason="softmax denom recip in f32r"):
                            nc.vector.reciprocal(rr[RD:RD + 1, :], ps2[RD:RD + 1, :CH])
                        rrd = rrd_p.tile([1, CH], F32R, tag="rrd", name="rrd")
                        nc.sync.dma_start(out=rrd[:], in_=rr[RD:RD + 1, :])
                        rb = rb_p.tile([128, CH], F32R, tag="rb", name="rb")
                        nc.sync.dma_start(
                            out=rb[:],
                            in_=bass.AP(tensor=rrd.tensor, offset=rrd.offset,
                                        ap=[[0, 128]] + list(rrd.ap[1:])))
                        nc.vector.tensor_mul(ota[:, h, :], ps1[:, :CH], rb[:])
                        nc.vector.tensor_mul(otb[:, h, :], ps2[0:RD, :CH], rb[0:RD, :])

                # ===== FINAL(c): out = attn @ WO, L-major (W-moving) =====
                for eg in range(E // CH):
                    woall = wo_p.tile([128, HPG, CH], F32R, tag="woall", name="woall")
                    nc.scalar.dma_start(out=woall[:], in_=woa.ap()[eg])
                    woallb = wo_p.tile([RD, HPG, CH], F32R, tag="woallb", name="woallb")
                    nc.scalar.dma_start(out=woallb[:], in_=wob.ap()[eg])
                    wts = [(woall[:, kt, :], woallb[:, kt, :]) for kt in range(HPG)]
                    for ls in range(CH // 128):
                        fps = psum()
                        for kt in range(HPG):
                            nc.tensor.matmul(fps[:, :CH], ota[:, kt, ds(128 * ls, 128)],
                                             wts[kt][0],
                                             start=(kt == 0), stop=False,
                                             skip_group_check=True)
                        for kt in range(HPG):
                            nc.tensor.matmul(fps[:, :CH], otb[:, kt, ds(128 * ls, 128)],
                                             wts[kt][1],
                                             start=False, stop=(kt == HPG - 1),
                                             skip_group_check=True)
                        fin = fin_p.tile([128, CH], F32, tag="fin", name="fin")
                        nc.scalar.copy(out=fin[:], in_=fps[:, :CH])
                        nc.gpsimd.dma_start(
                            out=outt.ap()[ds(c * CH + 128 * ls, 128), ds(CH * eg, CH)],
                            in_=fin[:])

    _split_excess_waits(nc)
    return nc


def _prep_inputs(x, cos_table, sin_table, wq, wkv_down, w_up, w_out):
    f32 = np.float32
    wq3 = np.asarray(wq, f32).reshape(E, H, DV)
    wup3 = np.asarray(w_up, f32).reshape(RK, H, 2 * DH + RD)
    wo3 = np.asarray(w_out, f32).reshape(H, DV, E)
    wkv = np.asarray(wkv_down, f32)

    cosI = np.repeat(np.asarray(cos_table, f32)[:L], 2, axis=1).T  # [64, L]
    sinI = np.repeat(np.asarray(sin_table, f32)[:L], 2, axis=1).T
    cost = np.ascontiguousarray(np.concatenate([cosI, cosI], 0))   # [128, L]
    sint = np.ascontiguousarray(np.concatenate([sinI, sinI], 0))
    J = np.zeros((128, 128), f32)
    for i in range(64):
        J[2 * i, 2 * i + 1] = -1.0
        J[2 * i + 1, 2 * i] = 1.0
    jt = np.ascontiguousarray(J.T)
    triu = np.ascontiguousarray(np.triu(np.ones((128, 128), f32)))

    in_maps = []
    for core in range(NCORE):
        b, g = core // HPG, core % HPG
        hs = slice(HPG * g, HPG * g + HPG)
        xT = np.asarray(x, f32)[b].T                       # [E, L]
        xt_pack = np.ascontiguousarray(
            xT.reshape(ET, 128, NCH, CH).transpose(2, 1, 0, 3))  # [NCH,128,ET,CH]
        wq_c = wq3[:, hs, :DH].reshape(E, HPG * DH)
        wq_r = wq3[:, hs, DH:].reshape(E, HPG * RD)
        w1_flat = np.concatenate([wq_c, wkv[:, :RK], wq_r, wkv[:, RK:]], axis=1)
        # pack into 11 d-strips [128, ET, 128] (last strip: 64 cols, zero-pad)
        woa_flat = wo3[hs, :DH, :].reshape(HPG, DH, E)     # [4,128,E]
        woa_pack = np.ascontiguousarray(
            woa_flat.reshape(HPG, DH, E // 512, 512).transpose(2, 1, 0, 3))
        wob_flat = wo3[hs, DH:, :].reshape(HPG, RD, E)
        wob_pack = np.ascontiguousarray(
            wob_flat.reshape(HPG, RD, E // 512, 512).transpose(2, 1, 0, 3))
        w1_pack = np.zeros((11, 128, ET, 128), f32)
        offs = [128 * i for i in range(10)] + [1280]
        wids = [128] * 10 + [64]
        for di, (o, w) in enumerate(zip(offs, wids)):
            w1_pack[di, :, :, :w] = (
                w1_flat[:, o:o + w].reshape(ET, 128, w).transpose(1, 0, 2))
        in_maps.append({
            "xt": xt_pack,
            "w1": np.ascontiguousarray(w1_pack),
            "wuk": np.ascontiguousarray(wup3[:, hs, :DH].reshape(RK, HPG * DH)),
            "wuv": np.ascontiguousarray(wup3[:, hs, DH:].reshape(RK, HPG * DV)),
            "woa": woa_pack,
            "wob": wob_pack,
            "cost": cost,
            "sint": sint,
            "jt": jt,
            "onesc": np.ones((128, HPG), f32),
            "triu": triu,
        })
    return in_maps


def kernel(x, cos_table, sin_table, wq, wkv_down, w_up, w_out, _want_perf=False):
    if "nc" not in _CACHE:
        _CACHE["nc"] = _build()
    nc = _CACHE["nc"]
    in_maps = _prep_inputs(x, cos_table, sin_table, wq, wkv_down, w_up, w_out)
    res = run_bass_kernel_spmd(nc, in_maps, core_ids=list(range(NCORE)),
                               trace=bool(_want_perf))
    out = np.zeros((B, L, E), np.float32)
    for core in range(NCORE):
        b = core // HPG
        out[b] += res.results[core]["outt"]
    if _want_perf:
        return out, res
    return out



# revision 12
# speedup vs baseline: 1.3509x; 1.3509x over previous
"""MLA prefill kernel for Trainium2, 8 NeuronCores.

Sharding: data-parallel over batch (2) x tensor-parallel over heads
(16 heads -> 4 per core).  Core c handles batch c//4, head group c%4.
Each core computes its full attention block plus a partial output
projection; the host sums the 4 per-group partials per batch.

v2 layout strategy (vs the f32r baseline):
- All compute tensors fp16 (PSUM accumulation stays fp32).  Halves
  DMA bytes and SBUF footprint; matmul stream rate is unchanged.
- All weights (fused QKV, up-proj, out-proj) are loaded once and kept
  resident in SBUF, so no mid-loop weight DMA and the PE never idles
  long enough for the HAM clock gate to re-throttle.
- V stays in SBUF for the whole kernel (24.1 KiB/partition) instead
  of spilling to a DRAM scratch.
- Scores are computed transposed (S^T = K Q^T, [Lk, Lq]) so softmax's
  sum runs through the PV matmul via an appended ones-column; exp for
  both heads of a pair is batched into one activation call over a
  2-bank PSUM tile.
- RoPE pair mixing runs along partitions via a +-1 pair-swap matmul.
"""

import math
import sys

sys.path.insert(0, "/opt/trn_rl_repo")

import numpy as np

import concourse.bass as bass
import concourse.mybir as mybir
import concourse.tile as tile
from concourse.bass import ds
from concourse.bass_utils import run_bass_kernel_spmd

H, DH, RK, RD = 16, 128, 512, 64
B, L, E = 2, 2048, 2048
HPG = 4                      # heads per core
NCORE = 8
DV = DH + RD                 # 192
SCALE = 1.0 / math.sqrt(DV)
CH = 512                     # Lq chunk
NCH = L // CH                # 4
LT = L // 128                # 16 key tiles
ET = E // 128                # 16
W1C = HPG * DH + RK + HPG * RD + RD   # 1344 fused QKV columns
VW = DV + 1                  # 193: per-head v dims + ones col

F16 = mybir.dt.float16
F32 = mybir.dt.float32
AF = mybir.ActivationFunctionType

_CACHE = {}


def _split_excess_waits(nc, limit=1):
    """walrus on this toolchain accepts at most one sem-wait per
    instruction; hoist extras onto same-engine no-ops just before."""
    f = nc.m.functions[0]
    for bb in f.blocks:
        new_list = []
        changed = False
        for inst in bb.instructions:
            si = inst.sync_info
            if si is not None and si.on_wait is not None and len(si.on_wait) > limit:
                waits = list(si.on_wait)
                changed = True
                n = 0
                while len(waits) > limit:
                    chunk, waits = waits[:limit], waits[limit:]
                    new_list.append(mybir.InstNoOp(
                        name=f"{inst.name}-ws{n}",
                        sync_info=mybir.SyncInfo(on_wait=chunk, on_update=[]),
                        bass_nofuse=True,
                        engine=inst.engine,
                    ))
                    n += 1
                inst.sync_info = mybir.SyncInfo(on_wait=waits, on_update=si.on_update)
            new_list.append(inst)
        if changed:
            bb.instructions[:] = new_list
    return nc


def _build():
    nc = bass.Bass(target_bir_lowering=False, trn_type="TRN2")

    xt = nc.dram_tensor("xt", [NCH, 128, ET, CH], F16, kind="ExternalInput")
    w1 = nc.dram_tensor("w1", [128, ET, W1C], F16, kind="ExternalInput")
    wuk = nc.dram_tensor("wuk", [RK, HPG * DH], F16, kind="ExternalInput")
    wuv = nc.dram_tensor("wuv", [RK, HPG * DV], F16, kind="ExternalInput")
    woa = nc.dram_tensor("woa", [E // CH, 128, HPG, CH], F16, kind="ExternalInput")
    wob = nc.dram_tensor("wob", [E // CH, RD, HPG, CH], F16, kind="ExternalInput")
    cost = nc.dram_tensor("cost", [128, L], F16, kind="ExternalInput")
    sint = nc.dram_tensor("sint", [128, L], F16, kind="ExternalInput")
    jt = nc.dram_tensor("jt", [128, 128], F16, kind="ExternalInput")
    triu = nc.dram_tensor("triu", [128, 128], F16, kind="ExternalInput")
    onesc = nc.dram_tensor("onesc", [128, LT * HPG], F16, kind="ExternalInput")
    outt = nc.dram_tensor("outt", [L, E], F16, kind="ExternalOutput")

    from contextlib import ExitStack

    with tile.TileContext(nc) as tc:
        with ExitStack() as ctx:
            pool_specs = [
                ("res", 1, None), ("rrd_p", 4, "DRAM"),
                ("xt_p", 2, None), ("qt_p", 2, None), ("rq_p", 2, None),
                ("ckv_p", 2, None), ("cs_p", 2, None), ("p_p", 2, None),
                ("tmp_p", 1, None), ("rr_p", 1, None),
                ("ot_p", 1, None), ("fin_p", 2, None), ("rb_p", 1, None),
                ("ps_p", 4, "PSUM"), ("ps2_p", 2, "PSUM"),
            ]
            pools = {}
            for pname, pbufs, pspace in pool_specs:
                kw = {"name": pname, "bufs": pbufs}
                if pspace:
                    kw["space"] = pspace
                pools[pname] = ctx.enter_context(tc.tile_pool(**kw))
            (res, rrd_p, xt_p, qt_p, rq_p, ckv_p, cs_p, p_p, tmp_p, rr_p,
             ot_p, fin_p, rb_p, ps_p, ps2_p) = (pools[s[0]] for s in pool_specs)

            def psum():
                return ps_p.tile([128, 512], F32, tag="ps", name="ps")

            # ---- resident constants / weights (one-time DMA) ----
            jtr = res.tile([128, 128], F16, tag="jtr", name="jtr")
            nc.sync.dma_start(out=jtr[:], in_=jt.ap())
            trir = res.tile([128, 128], F16, tag="trir", name="trir")
            nc.sync.dma_start(out=trir[:], in_=triu.ap())
            w1r = res.tile([128, ET, W1C], F16, tag="w1r", name="w1r")
            nc.sync.dma_start(out=w1r[:], in_=w1.ap())
            wukt = res.tile([128, RK // 128, HPG * DH], F16, tag="wukt", name="wukt")
            nc.sync.dma_start(out=wukt[:], in_=wuk.ap().rearrange("(t p) n -> p t n", p=128))
            wuvt = res.tile([128, RK // 128, HPG * DV], F16, tag="wuvt", name="wuvt")
            nc.sync.dma_start(out=wuvt[:], in_=wuv.ap().rearrange("(t p) n -> p t n", p=128))
            woar = res.tile([128, E // CH, HPG, CH], F16, tag="woar", name="woar")
            nc.sync.dma_start(out=woar[:], in_=woa.ap().rearrange("g p h f -> p g h f"))
            wobr = res.tile([RD, E // CH, HPG, CH], F16, tag="wobr", name="wobr")
            nc.sync.dma_start(out=wobr[:], in_=wob.ap().rearrange("g p h f -> p g h f"))

            ktc = res.tile([128, HPG, L], F16, tag="ktc", name="ktc")   # K content^T
            rkd = res.tile([128, L], F16, tag="rkd", name="rkd")        # roped k_rope, dup rows
            vsb = res.tile([128, LT, HPG * VW], F16, tag="vsb", name="vsb")  # V resident

            # ones columns of V (per tile t, per head: col h*VW + DV)
            ones_view = vsb[:].rearrange("p t (h x) -> p (t h) x", x=VW)
            nc.sync.dma_start(out=ones_view[:, :, DV], in_=onesc.ap())

            # d-strips of the fused QKV projection: (offset, width, kind, idx)
            dtiles = (
                [(128 * i, 128, "q", i) for i in range(HPG)]
                + [(HPG * DH + 128 * i, 128, "ckv", i) for i in range(RK // 128)]
                + [(HPG * DH + RK + 128 * i, 128, "rq", i) for i in range(2)]
                + [(HPG * DH + RK + HPG * RD, RD, "rk", 0)]
            )

            for c in range(NCH):
                ccols = ds(c * CH, CH)

                # ================= QKV(c): [1344, CH] = W1^T @ x^T =======
                xtt = xt_p.tile([128, ET, CH], F16, tag="xtt", name="xtt")
                nc.sync.dma_start(out=xtt[:], in_=xt.ap()[c])
                cs = cs_p.tile([128, 2, CH], F16, tag="cs", name="cs")
                nc.sync.dma_start(out=cs[:, 0, :], in_=cost.ap()[:, ccols])
                nc.sync.dma_start(out=cs[:, 1, :], in_=sint.ap()[:, ccols])
                qtc = qt_p.tile([128, HPG, CH], F16, tag="qtc", name="qtc")
                rq = rq_p.tile([128, 2, CH], F16, tag="rq", name="rq")
                ckv = ckv_p.tile([128, RK // 128, CH], F16, tag="ckv", name="ckv")

                for doff, dw, kind, idx in dtiles:
                    ps = psum()
                    for e in range(ET):
                        nc.tensor.matmul(ps[:dw, :CH], w1r[:, e, ds(doff, dw)],
                                         xtt[:, e, :],
                                         start=(e == 0), stop=(e == ET - 1))
                    if kind == "q":
                        nc.scalar.copy(out=qtc[:, idx, :], in_=ps[:, :CH])
                    elif kind == "ckv":
                        nc.vector.tensor_copy(ckv[:, idx, :], ps[:, :CH])
                    elif kind == "rq":
                        nc.vector.tensor_copy(rq[:, idx, :], ps[:, :CH])
                    else:  # pre-rope k_rope at partitions 0:64
                        nc.vector.tensor_copy(rkd[0:RD, ccols], ps[:RD, :CH])

                # ================= RoPE(c) ===============================
                # roped = R * cos + (J @ R) * sin   (pairs along partitions)
                for i in range(2):  # q_rope, two head-pair tiles
                    swp = psum()
                    nc.tensor.matmul(swp[:, :CH], jtr[:, :], rq[:, i, :],
                                     start=True, stop=True)
                    swpc = tmp_p.tile([128, CH], F16, tag="swpc", name="swpc")
                    nc.scalar.copy(out=swpc[:], in_=swp[:, :CH])
                    t1 = tmp_p.tile([128, CH], F16, tag="ropet", name="ropet")
                    nc.vector.tensor_mul(t1[:], rq[:, i, :], cs[:, 0, :])
                    nc.vector.tensor_mul(rq[:, i, :], swpc[:], cs[:, 1, :])
                    nc.vector.tensor_add(rq[:, i, :], rq[:, i, :], t1[:])
                swp = psum()
                nc.tensor.matmul(swp[:RD, :CH], jtr[:RD, :RD], rkd[0:RD, ccols],
                                 start=True, stop=True)
                swpc = tmp_p.tile([128, CH], F16, tag="swpc", name="swpc")
                nc.scalar.copy(out=swpc[:RD, :], in_=swp[:RD, :CH])
                t1 = tmp_p.tile([128, CH], F16, tag="ropet", name="ropet")
                nc.vector.tensor_mul(t1[:RD, :], rkd[0:RD, ccols], cs[0:RD, 0, :])
                nc.vector.tensor_mul(rkd[0:RD, ccols], swpc[:RD, :], cs[0:RD, 1, :])
                nc.vector.tensor_add(rkd[0:RD, ccols], rkd[0:RD, ccols], t1[:RD, :])
                # duplicate roped k_rope to partitions 64:128 (for odd heads)
                nc.sync.dma_start(out=rkd[RD:128, ccols], in_=rkd[0:RD, ccols])

                # ================= UP-K(c): K^T = Wuk^T @ c_kv^T =========
                for h in range(HPG):
                    ps = psum()
                    for kt in range(RK // 128):
                        nc.tensor.matmul(ps[:, :CH], wukt[:, kt, ds(128 * h, 128)],
                                         ckv[:, kt, :],
                                         start=(kt == 0), stop=(kt == RK // 128 - 1))
                    nc.scalar.copy(out=ktc[:, h, ccols], in_=ps[:, :CH])

                # ================= UP-V(c): V = c_kv @ Wuv (L-major) =====
                for lti in range(4):
                    lt = 4 * c + lti
                    for nb in range(2):
                        psv = psum()
                        for kt in range(RK // 128):
                            nc.tensor.matmul(psv[:, :384],
                                             ckv[:, kt, ds(128 * lti, 128)],
                                             wuvt[:, kt, ds(384 * nb, 384)],
                                             start=(kt == 0), stop=(kt == RK // 128 - 1))
                        for q in range(2):
                            hh = 2 * nb + q
                            nc.scalar.copy(out=vsb[:, lt, ds(VW * hh, DV)],
                                           in_=psv[:, ds(DV * q, DV)])

                # ================= ATT(c): head pairs ====================
                ntk = 4 * c + 4
                for hp in range(2):
                    pvs = []
                    for q in range(2):
                        pvs.append((psum(), psum()))  # (ps1, ps2) per head
                    for t in range(ntk):
                        j = t - 4 * c
                        off = 128 * j if j >= 0 else 0
                        n = CH - off
                        sps2 = ps2_p.tile([128, 2, 512], F32, tag="sps2", name="sps2")
                        for q in range(2):
                            h = 2 * hp + q
                            hb = RD * (h % 2)
                            nc.tensor.matmul(sps2[:, q, ds(off, n)],
                                             ktc[:, h, ds(128 * t, 128)],
                                             qtc[:, h, ds(off, n)],
                                             start=True, stop=False)
                            nc.tensor.matmul(sps2[:, q, ds(off, n)],
                                             rkd[hb:hb + RD, ds(128 * t, 128)],
                                             rq[hb:hb + RD, hp, ds(off, n)],
                                             start=False, stop=True)
                        pt2 = p_p.tile([128, 2, CH], F16, tag="pt2", name="pt2")
                        nc.scalar.activation(pt2[:, :, ds(off, n)],
                                             sps2[:, :, ds(off, n)],
                                             AF.Exp, scale=SCALE)
                        if j >= 0:
                            for q in range(2):
                                nc.vector.tensor_mul(pt2[:, q, ds(off, 128)],
                                                     pt2[:, q, ds(off, 128)], trir[:])
                        for q in range(2):
                            h = 2 * hp + q
                            ps1, ps2 = pvs[q]
                            nc.tensor.matmul(ps1[:, ds(off, n)],
                                             vsb[:, t, ds(VW * h, 128)],
                                             pt2[:, q, ds(off, n)],
                                             start=(t == 0), stop=(t == ntk - 1),
                                             skip_group_check=True)
                            nc.tensor.matmul(ps2[:VW - DH, ds(off, n)],
                                             vsb[:, t, ds(VW * h + DH, VW - DH)],
                                             pt2[:, q, ds(off, n)],
                                             start=(t == 0), stop=(t == ntk - 1),
                                             skip_group_check=True)
                    if hp == 0:
                        ota = ot_p.tile([128, HPG, CH], F16, tag="ota", name="ota")
                        otb = ot_p.tile([RD, HPG, CH], F16, tag="otb", name="otb")
                    for q in range(2):
                        h = 2 * hp + q
                        ps1, ps2 = pvs[q]
                        rr = rr_p.tile([128, CH], F32, tag="rr", name="rr")
                        nc.vector.reciprocal(rr[RD:RD + 1, :], ps2[RD:RD + 1, :CH])
                        rrd = rrd_p.tile([1, CH], F32, tag="rrd", name="rrd")
                        nc.sync.dma_start(out=rrd[:], in_=rr[RD:RD + 1, :])
                        rb = rb_p.tile([128, CH], F32, tag="rb", name="rb")
                        nc.sync.dma_start(
                            out=rb[:],
                            in_=bass.AP(tensor=rrd.tensor, offset=rrd.offset,
                                        ap=[[0, 128]] + list(rrd.ap[1:])))
                        nc.vector.tensor_mul(ota[:, h, :], ps1[:, :CH], rb[:])
                        nc.vector.tensor_mul(otb[:, h, :], ps2[0:RD, :CH], rb[0:RD, :])

                # ===== FINAL(c): out = attn @ WO, L-major (W-moving) =====
                for eg in range(E // CH):
                    for ls in range(CH // 128):
                        fps = psum()
                        for kt in range(HPG):
                            nc.tensor.matmul(fps[:, :CH], ota[:, kt, ds(128 * ls, 128)],
                                             woar[:, eg, kt, :],
                                             start=(kt == 0), stop=False,
                                             skip_group_check=True)
                        for kt in range(HPG):
                            nc.tensor.matmul(fps[:, :CH], otb[:, kt, ds(128 * ls, 128)],
                                             wobr[:, eg, kt, :],
                                             start=False, stop=(kt == HPG - 1),
                                             skip_group_check=True)
                        fin = fin_p.tile([128, CH], F16, tag="fin", name="fin")
                        if ls % 2 == 0:
                            nc.scalar.copy(out=fin[:], in_=fps[:, :CH])
                        else:
                            nc.vector.tensor_copy(fin[:], fps[:, :CH])
                        nc.sync.dma_start(
                            out=outt.ap()[ds(c * CH + 128 * ls, 128), ds(CH * eg, CH)],
                            in_=fin[:])

    _split_excess_waits(nc)
    return nc


def _prep_inputs(x, cos_table, sin_table, wq, wkv_down, w_up, w_out):
    f16 = np.float16
    f32 = np.float32
    wq3 = np.asarray(wq, f32).reshape(E, H, DV)
    wup3 = np.asarray(w_up, f32).reshape(RK, H, 2 * DH + RD)
    wo3 = np.asarray(w_out, f32).reshape(H, DV, E)
    wkv = np.asarray(wkv_down, f32)

    cosI = np.repeat(np.asarray(cos_table, f32)[:L], 2, axis=1).T  # [64, L]
    sinI = np.repeat(np.asarray(sin_table, f32)[:L], 2, axis=1).T
    cost = np.ascontiguousarray(np.concatenate([cosI, cosI], 0)).astype(f16)
    sint = np.ascontiguousarray(np.concatenate([sinI, sinI], 0)).astype(f16)
    J = np.zeros((128, 128), f32)
    for i in range(64):
        J[2 * i, 2 * i + 1] = -1.0
        J[2 * i + 1, 2 * i] = 1.0
    jt = np.ascontiguousarray(J.T).astype(f16)
    triu = np.ascontiguousarray(np.triu(np.ones((128, 128), f32))).astype(f16)

    in_maps = []
    for core in range(NCORE):
        b, g = core // HPG, core % HPG
        hs = slice(HPG * g, HPG * g + HPG)
        xT = np.asarray(x, f32)[b].T                       # [E, L]
        xt_pack = np.ascontiguousarray(
            xT.reshape(ET, 128, NCH, CH).transpose(2, 1, 0, 3)).astype(f16)
        wq_c = wq3[:, hs, :DH].reshape(E, HPG * DH)
        wq_r = wq3[:, hs, DH:].reshape(E, HPG * RD)
        w1_flat = np.concatenate([wq_c, wkv[:, :RK], wq_r, wkv[:, RK:]], axis=1)
        woa_flat = wo3[hs, :DH, :].reshape(HPG, DH, E)     # [4,128,E]
        woa_pack = np.ascontiguousarray(
            woa_flat.reshape(HPG, DH, E // CH, CH).transpose(2, 1, 0, 3)).astype(f16)
        wob_flat = wo3[hs, DH:, :].reshape(HPG, RD, E)
        wob_pack = np.ascontiguousarray(
            wob_flat.reshape(HPG, RD, E // CH, CH).transpose(2, 1, 0, 3)).astype(f16)
        w1_pack = np.ascontiguousarray(
            w1_flat.reshape(ET, 128, W1C).transpose(1, 0, 2)).astype(f16)
        in_maps.append({
            "xt": xt_pack,
            "w1": np.ascontiguousarray(w1_pack),
            "wuk": np.ascontiguousarray(
                wup3[:, hs, :DH].reshape(RK, HPG * DH)).astype(f16),
            "wuv": np.ascontiguousarray(
                wup3[:, hs, DH:].reshape(RK, HPG * DV)).astype(f16),
            "woa": woa_pack,
            "wob": wob_pack,
            "cost": cost,
            "sint": sint,
            "jt": jt,
            "onesc": np.ones((128, LT * HPG), f16),
            "triu": triu,
        })
    return in_maps


def kernel(x, cos_table, sin_table, wq, wkv_down, w_up, w_out, _want_perf=False):
    if "nc" not in _CACHE:
        _CACHE["nc"] = _build()
    nc = _CACHE["nc"]
    in_maps = _prep_inputs(x, cos_table, sin_table, wq, wkv_down, w_up, w_out)
    res = run_bass_kernel_spmd(nc, in_maps, core_ids=list(range(NCORE)),
                               trace=bool(_want_perf))
    out = np.zeros((B, L, E), np.float32)
    for core in range(NCORE):
        b = core // HPG
        out[b] += res.results[core]["outt"].astype(np.float32)
    if _want_perf:
        return out, res
    return out


# revision 16
# speedup vs baseline: 2.0245x; 1.4987x over previous
"""MLA prefill kernel for Trainium2, 8 NeuronCores.

Sharding: data-parallel over batch (2) x tensor-parallel over heads
(16 heads -> 4 per core).  Core c handles batch c//4, head group c%4.
Each core computes its full attention block plus a partial output
projection; the host sums the 4 per-group partials per batch.

v2 layout strategy (vs the f32r baseline):
- All compute tensors fp16 (PSUM accumulation stays fp32).  Halves
  DMA bytes and SBUF footprint; matmul stream rate is unchanged.
- All weights (fused QKV, up-proj, out-proj) are loaded once and kept
  resident in SBUF, so no mid-loop weight DMA and the PE never idles
  long enough for the HAM clock gate to re-throttle.
- V stays in SBUF for the whole kernel (24.1 KiB/partition) instead
  of spilling to a DRAM scratch.
- Scores are computed transposed (S^T = K Q^T, [Lk, Lq]) so softmax's
  sum runs through the PV matmul via an appended ones-column; exp for
  both heads of a pair is batched into one activation call over a
  2-bank PSUM tile.
- RoPE pair mixing runs along partitions via a +-1 pair-swap matmul.
"""

import math
import sys

sys.path.insert(0, "/opt/trn_rl_repo")

import numpy as np

import concourse.bass as bass
import concourse.mybir as mybir
import concourse.tile as tile
from concourse.bass import ds
from concourse.bass_utils import run_bass_kernel_spmd

H, DH, RK, RD = 16, 128, 512, 64
B, L, E = 2, 2048, 2048
HPG = 4                      # heads per core
NCORE = 8
DV = DH + RD                 # 192
SCALE = 1.0 / math.sqrt(DV)
CH = 512                     # Lq chunk
NCH = L // CH                # 4
LT = L // 128                # 16 key tiles
ET = E // 128                # 16
W1C = HPG * DH + RK + HPG * RD + RD   # 1344 fused QKV columns
VW = DV + 1                  # 193: per-head v dims + ones col

F16 = mybir.dt.float16
F32 = mybir.dt.float32
AF = mybir.ActivationFunctionType

_CACHE = {}


def _split_excess_waits(nc, limit=1):
    """walrus on this toolchain accepts at most one sem-wait per
    instruction; hoist extras onto same-engine no-ops just before."""
    f = nc.m.functions[0]
    for bb in f.blocks:
        new_list = []
        changed = False
        for inst in bb.instructions:
            si = inst.sync_info
            if si is not None and si.on_wait is not None and len(si.on_wait) > limit:
                waits = list(si.on_wait)
                changed = True
                n = 0
                while len(waits) > limit:
                    chunk, waits = waits[:limit], waits[limit:]
                    new_list.append(mybir.InstNoOp(
                        name=f"{inst.name}-ws{n}",
                        sync_info=mybir.SyncInfo(on_wait=chunk, on_update=[]),
                        bass_nofuse=True,
                        engine=inst.engine,
                    ))
                    n += 1
                inst.sync_info = mybir.SyncInfo(on_wait=waits, on_update=si.on_update)
            new_list.append(inst)
        if changed:
            bb.instructions[:] = new_list
    return nc


def _build():
    nc = bass.Bass(target_bir_lowering=False, trn_type="TRN2")

    xt = nc.dram_tensor("xt", [NCH, 128, ET, CH], F16, kind="ExternalInput")
    w1 = nc.dram_tensor("w1", [128, ET, W1C], F16, kind="ExternalInput")
    wuk = nc.dram_tensor("wuk", [RK, HPG * DH], F16, kind="ExternalInput")
    wuv = nc.dram_tensor("wuv", [RK, HPG * DV], F16, kind="ExternalInput")
    woa = nc.dram_tensor("woa", [E // CH, 128, HPG, CH], F16, kind="ExternalInput")
    wob = nc.dram_tensor("wob", [E // CH, RD, HPG, CH], F16, kind="ExternalInput")
    cost = nc.dram_tensor("cost", [128, L], F16, kind="ExternalInput")
    sint = nc.dram_tensor("sint", [128, L], F16, kind="ExternalInput")
    jt = nc.dram_tensor("jt", [128, 128], F16, kind="ExternalInput")
    triu = nc.dram_tensor("triu", [128, 128], F16, kind="ExternalInput")
    onesc = nc.dram_tensor("onesc", [128, LT * HPG], F16, kind="ExternalInput")
    outt = nc.dram_tensor("outt", [L, E], F16, kind="ExternalOutput")

    from contextlib import ExitStack

    with tile.TileContext(nc) as tc:
        with ExitStack() as ctx:
            pool_specs = [
                ("res", 1, None), ("rrd_p", 4, "DRAM"),
                ("xt_p", 2, None), ("qt_p", 2, None), ("rq_p", 2, None),
                ("ckv_p", 2, None), ("cs_p", 1, None), ("p_p", 2, None),
                ("tmp_p", 1, None), ("rr_p", 2, None),
                ("ot_p", 1, None), ("uo_p", 1, None),
                ("fin_p", 2, None), ("rb_p", 2, None),
                ("ps_p", 4, "PSUM"), ("ps2_p", 2, "PSUM"),
            ]
            pools = {}
            for pname, pbufs, pspace in pool_specs:
                kw = {"name": pname, "bufs": pbufs}
                if pspace:
                    kw["space"] = pspace
                pools[pname] = ctx.enter_context(tc.tile_pool(**kw))
            (res, rrd_p, xt_p, qt_p, rq_p, ckv_p, cs_p, p_p, tmp_p, rr_p,
             ot_p, uo_p, fin_p, rb_p, ps_p, ps2_p) = (pools[s[0]] for s in pool_specs)

            def psum():
                return ps_p.tile([128, 512], F32, tag="ps", name="ps")

            # ---- resident constants / weights (one-time DMA) ----
            jtr = res.tile([128, 128], F16, tag="jtr", name="jtr")
            nc.sync.dma_start(out=jtr[:], in_=jt.ap())
            trir = res.tile([128, 128], F16, tag="trir", name="trir")
            nc.sync.dma_start(out=trir[:], in_=triu.ap())
            # w1 in 4 e-strips so chunk-0 QKV starts before the full load lands
            w1r = res.tile([128, ET, W1C], F16, tag="w1r", name="w1r")
            for es in range(4):
                nc.sync.dma_start(out=w1r[:, ds(4 * es, 4), :],
                                  in_=w1.ap()[:, ds(4 * es, 4), :])
            wukt = res.tile([128, RK // 128, HPG * DH], F16, tag="wukt", name="wukt")
            nc.sync.dma_start(out=wukt[:], in_=wuk.ap().rearrange("(t p) n -> p t n", p=128))
            wuvt = res.tile([128, RK // 128, HPG * DV], F16, tag="wuvt", name="wuvt")
            nc.sync.dma_start(out=wuvt[:], in_=wuv.ap().rearrange("(t p) n -> p t n", p=128))
            woar = res.tile([128, E // CH, HPG, CH], F16, tag="woar", name="woar")
            nc.sync.dma_start(out=woar[:], in_=woa.ap().rearrange("g p h f -> p g h f"))
            wobr = res.tile([RD, E // CH, HPG, CH], F16, tag="wobr", name="wobr")
            nc.sync.dma_start(out=wobr[:], in_=wob.ap().rearrange("g p h f -> p g h f"))

            ktc = res.tile([128, HPG, L], F16, tag="ktc", name="ktc")   # K content^T
            rkd = res.tile([128, L], F16, tag="rkd", name="rkd")        # roped k_rope, dup rows
            vsb = res.tile([128, LT, HPG * VW], F16, tag="vsb", name="vsb")  # V resident

            # ones columns of V (per tile t, per head: col h*VW + DV)
            ones_view = vsb[:].rearrange("p t (h x) -> p (t h) x", x=VW)
            nc.sync.dma_start(out=ones_view[:, :, DV], in_=onesc.ap())

            # d-strips of the fused QKV projection: (offset, width, kind, idx)
            dtiles = (
                [(128 * i, 128, "q", i) for i in range(HPG)]
                + [(HPG * DH + 128 * i, 128, "ckv", i) for i in range(RK // 128)]
                + [(HPG * DH + RK + 128 * i, 128, "rq", i) for i in range(2)]
                + [(HPG * DH + RK + HPG * RD, RD, "rk", 0)]
            )

            def emit_qkv(c):
                ccols = ds(c * CH, CH)
                # x^T chunk + rope tables via SWDGE so they don't queue
                # behind the sync ring's resident/output transfers.
                xtt = xt_p.tile([128, ET, CH], F16, tag="xtt", name="xtt")
                nc.gpsimd.dma_start(out=xtt[:], in_=xt.ap()[c])
                cs = cs_p.tile([128, 2, CH], F16, tag="cs", name="cs")
                nc.gpsimd.dma_start(out=cs[:, 0, :], in_=cost.ap()[:, ccols])
                nc.gpsimd.dma_start(out=cs[:, 1, :], in_=sint.ap()[:, ccols])
                qtc = qt_p.tile([128, HPG, CH], F16, tag="qtc", name="qtc")
                rq = rq_p.tile([128, 2, CH], F16, tag="rq", name="rq")
                ckv = ckv_p.tile([128, RK // 128, CH], F16, tag="ckv", name="ckv")

                for doff, dw, kind, idx in dtiles:
                    ps = psum()
                    for e in range(ET):
                        nc.tensor.matmul(ps[:dw, :CH], w1r[:, e, ds(doff, dw)],
                                         xtt[:, e, :],
                                         start=(e == 0), stop=(e == ET - 1))
                    if kind == "q":
                        nc.scalar.copy(out=qtc[:, idx, :], in_=ps[:, :CH])
                    elif kind == "ckv":
                        nc.vector.tensor_copy(ckv[:, idx, :], ps[:, :CH])
                    elif kind == "rq":
                        nc.vector.tensor_copy(rq[:, idx, :], ps[:, :CH])
                    else:  # pre-rope k_rope at partitions 0:64
                        nc.vector.tensor_copy(rkd[0:RD, ccols], ps[:RD, :CH])
                return cs, qtc, rq, ckv

            def emit_rope(c, cs, rq):
                ccols = ds(c * CH, CH)
                # roped = R * cos + (J @ R) * sin   (pairs along partitions)
                for i in range(2):  # q_rope, two head-pair tiles
                    swp = psum()
                    nc.tensor.matmul(swp[:, :CH], jtr[:, :], rq[:, i, :],
                                     start=True, stop=True)
                    t1 = tmp_p.tile([128, CH], F16, tag="ropet", name="ropet")
                    nc.vector.tensor_mul(t1[:], rq[:, i, :], cs[:, 0, :])
                    nc.vector.tensor_mul(rq[:, i, :], swp[:, :CH], cs[:, 1, :])
                    nc.vector.tensor_add(rq[:, i, :], rq[:, i, :], t1[:])
                swp = psum()
                nc.tensor.matmul(swp[:RD, :CH], jtr[:RD, :RD], rkd[0:RD, ccols],
                                 start=True, stop=True)
                t1 = tmp_p.tile([128, CH], F16, tag="ropet", name="ropet")
                nc.vector.tensor_mul(t1[:RD, :], rkd[0:RD, ccols], cs[0:RD, 0, :])
                nc.vector.tensor_mul(rkd[0:RD, ccols], swp[:RD, :CH], cs[0:RD, 1, :])
                nc.vector.tensor_add(rkd[0:RD, ccols], rkd[0:RD, ccols], t1[:RD, :])
                # duplicate roped k_rope to partitions 64:128 (for odd heads)
                nc.sync.dma_start(out=rkd[RD:128, ccols], in_=rkd[0:RD, ccols])

            def emit_upk(c, ckv):
                ccols = ds(c * CH, CH)
                for h in range(HPG):
                    ps = psum()
                    for kt in range(RK // 128):
                        nc.tensor.matmul(ps[:, :CH], wukt[:, kt, ds(128 * h, 128)],
                                         ckv[:, kt, :],
                                         start=(kt == 0), stop=(kt == RK // 128 - 1))
                    nc.scalar.copy(out=ktc[:, h, ccols], in_=ps[:, :CH])

            def emit_upv(c, ckv):
                for lti in range(4):
                    lt = 4 * c + lti
                    for nb in range(2):
                        psv = psum()
                        for kt in range(RK // 128):
                            nc.tensor.matmul(psv[:, :384],
                                             ckv[:, kt, ds(128 * lti, 128)],
                                             wuvt[:, kt, ds(384 * nb, 384)],
                                             start=(kt == 0), stop=(kt == RK // 128 - 1))
                        for q in range(2):
                            hh = 2 * nb + q
                            nc.scalar.copy(out=vsb[:, lt, ds(VW * hh, DV)],
                                           in_=psv[:, ds(DV * q, DV)])

            def emit_att(c, qtc, rq):
                ntk = 4 * c + 4
                ota = ot_p.tile([128, HPG, CH], F16, tag="ota", name="ota")
                otb = ot_p.tile([RD, HPG, CH], F16, tag="otb", name="otb")
                for hp in range(2):
                    pvs = []
                    for q in range(2):
                        pvs.append((psum(), psum()))  # (ps1, ps2) per head
                    for t in range(ntk):
                        j = t - 4 * c
                        off = 128 * j if j >= 0 else 0
                        n = CH - off
                        sps2 = ps2_p.tile([128, 2, 512], F32, tag="sps2", name="sps2")
                        # content scores for both heads, then the two K=64
                        # rope matmuls back-to-back: they sit in disjoint PE
                        # row-groups (base partitions 0 and 64) and run
                        # concurrently in the array.
                        for q in range(2):
                            h = 2 * hp + q
                            nc.tensor.matmul(sps2[:, q, ds(off, n)],
                                             ktc[:, h, ds(128 * t, 128)],
                                             qtc[:, h, ds(off, n)],
                                             start=True, stop=False,
                                             skip_group_check=True)
                        for q in range(2):
                            h = 2 * hp + q
                            hb = RD * (h % 2)
                            nc.tensor.matmul(sps2[:, q, ds(off, n)],
                                             rkd[hb:hb + RD, ds(128 * t, 128)],
                                             rq[hb:hb + RD, hp, ds(off, n)],
                                             start=False, stop=True,
                                             skip_group_check=True)
                        pt2 = p_p.tile([128, 2, CH], F16, tag="pt2", name="pt2")
                        nc.scalar.activation(pt2[:, :, ds(off, n)],
                                             sps2[:, :, ds(off, n)],
                                             AF.Exp, scale=SCALE)
                        if j >= 0:
                            for q in range(2):
                                nc.vector.tensor_mul(pt2[:, q, ds(off, 128)],
                                                     pt2[:, q, ds(off, 128)], trir[:])
                        for q in range(2):
                            h = 2 * hp + q
                            ps1, ps2 = pvs[q]
                            nc.tensor.matmul(ps1[:, ds(off, n)],
                                             vsb[:, t, ds(VW * h, 128)],
                                             pt2[:, q, ds(off, n)],
                                             start=(t == 0), stop=(t == ntk - 1),
                                             skip_group_check=True)
                            nc.tensor.matmul(ps2[:VW - DH, ds(off, n)],
                                             vsb[:, t, ds(VW * h + DH, VW - DH)],
                                             pt2[:, q, ds(off, n)],
                                             start=(t == 0), stop=(t == ntk - 1),
                                             skip_group_check=True)
                    uo = uo_p.tile([128, 2, CH], F16, tag="uo", name="uo")
                    for q in range(2):
                        h = 2 * hp + q
                        ps1, ps2 = pvs[q]
                        # evacuate ps1 immediately so its PSUM bank frees
                        # before the reciprocal broadcast round-trip lands
                        nc.scalar.copy(out=uo[:, q, :], in_=ps1[:, :CH])
                        rr = rr_p.tile([128, CH], F16, tag="rr", name="rr")
                        with nc.allow_low_precision(reason="softmax denom recip fp16"):
                            nc.vector.reciprocal(rr[RD:RD + 1, :], ps2[RD:RD + 1, :CH])
                        rrd = rrd_p.tile([1, CH], F16, tag="rrd", name="rrd")
                        nc.sync.dma_start(out=rrd[:], in_=rr[RD:RD + 1, :])
                        rb = rb_p.tile([128, CH], F16, tag="rb", name="rb")
                        nc.sync.dma_start(
                            out=rb[:],
                            in_=bass.AP(tensor=rrd.tensor, offset=rrd.offset,
                                        ap=[[0, 128]] + list(rrd.ap[1:])))
                        nc.vector.tensor_mul(ota[:, h, :], uo[:, q, :], rb[:])
                        nc.vector.tensor_mul(otb[:, h, :], ps2[0:RD, :CH],
                                             rb[0:RD, :])
                return ota, otb

            def emit_final(c, ota, otb):
                # out = attn @ WO, L-major (W-moving)
                for eg in range(E // CH):
                    for ls in range(CH // 128):
                        fps = psum()
                        for kt in range(HPG):
                            nc.tensor.matmul(fps[:, :CH], ota[:, kt, ds(128 * ls, 128)],
                                             woar[:, eg, kt, :],
                                             start=(kt == 0), stop=False,
                                             skip_group_check=True)
                        for kt in range(HPG):
                            nc.tensor.matmul(fps[:, :CH], otb[:, kt, ds(128 * ls, 128)],
                                             wobr[:, eg, kt, :],
                                             start=False, stop=(kt == HPG - 1),
                                             skip_group_check=True)
                        fin = fin_p.tile([128, CH], F16, tag="fin", name="fin")
                        if ls % 2 == 0:
                            nc.scalar.copy(out=fin[:], in_=fps[:, :CH])
                        else:
                            nc.vector.tensor_copy(fin[:], fps[:, :CH])
                        nc.sync.dma_start(
                            out=outt.ap()[ds(c * CH + 128 * ls, 128), ds(CH * eg, CH)],
                            in_=fin[:])

            # Chunk pipeline: FINAL(c) is emitted after QKV(c+1) so the
            # softmax-denominator broadcast round-trip of ATT(c) overlaps
            # the dense QKV matmuls instead of stalling the PE (and the
            # HAM clock gate never sees an idle window).
            prev_final = None
            for c in range(NCH):
                cs, qtc, rq, ckv = emit_qkv(c)
                if prev_final is not None:
                    emit_final(*prev_final)
                emit_rope(c, cs, rq)
                emit_upk(c, ckv)
                emit_upv(c, ckv)
                ota, otb = emit_att(c, qtc, rq)
                prev_final = (c, ota, otb)
            emit_final(*prev_final)

    _split_excess_waits(nc)
    return nc


def _prep_inputs(x, cos_table, sin_table, wq, wkv_down, w_up, w_out):
    f16 = np.float16
    f32 = np.float32
    wq3 = np.asarray(wq, f32).reshape(E, H, DV)
    wup3 = np.asarray(w_up, f32).reshape(RK, H, 2 * DH + RD)
    wo3 = np.asarray(w_out, f32).reshape(H, DV, E)
    wkv = np.asarray(wkv_down, f32)

    cosI = np.repeat(np.asarray(cos_table, f32)[:L], 2, axis=1).T  # [64, L]
    sinI = np.repeat(np.asarray(sin_table, f32)[:L], 2, axis=1).T
    cost = np.ascontiguousarray(np.concatenate([cosI, cosI], 0)).astype(f16)
    sint = np.ascontiguousarray(np.concatenate([sinI, sinI], 0)).astype(f16)
    J = np.zeros((128, 128), f32)
    for i in range(64):
        J[2 * i, 2 * i + 1] = -1.0
        J[2 * i + 1, 2 * i] = 1.0
    jt = np.ascontiguousarray(J.T).astype(f16)
    triu = np.ascontiguousarray(np.triu(np.ones((128, 128), f32))).astype(f16)

    in_maps = []
    for core in range(NCORE):
        b, g = core // HPG, core % HPG
        hs = slice(HPG * g, HPG * g + HPG)
        xT = np.asarray(x, f32)[b].T                       # [E, L]
        xt_pack = np.ascontiguousarray(
            xT.reshape(ET, 128, NCH, CH).transpose(2, 1, 0, 3)).astype(f16)
        wq_c = wq3[:, hs, :DH].reshape(E, HPG * DH)
        wq_r = wq3[:, hs, DH:].reshape(E, HPG * RD)
        w1_flat = np.concatenate([wq_c, wkv[:, :RK], wq_r, wkv[:, RK:]], axis=1)
        woa_flat = wo3[hs, :DH, :].reshape(HPG, DH, E)     # [4,128,E]
        woa_pack = np.ascontiguousarray(
            woa_flat.reshape(HPG, DH, E // CH, CH).transpose(2, 1, 0, 3)).astype(f16)
        wob_flat = wo3[hs, DH:, :].reshape(HPG, RD, E)
        wob_pack = np.ascontiguousarray(
            wob_flat.reshape(HPG, RD, E // CH, CH).transpose(2, 1, 0, 3)).astype(f16)
        w1_pack = np.ascontiguousarray(
            w1_flat.reshape(ET, 128, W1C).transpose(1, 0, 2)).astype(f16)
        in_maps.append({
            "xt": xt_pack,
            "w1": np.ascontiguousarray(w1_pack),
            "wuk": np.ascontiguousarray(
                wup3[:, hs, :DH].reshape(RK, HPG * DH)).astype(f16),
            "wuv": np.ascontiguousarray(
                wup3[:, hs, DH:].reshape(RK, HPG * DV)).astype(f16),
            "woa": woa_pack,
            "wob": wob_pack,
            "cost": cost,
            "sint": sint,
            "jt": jt,
            "onesc": np.ones((128, LT * HPG), f16),
            "triu": triu,
        })
    return in_maps


def kernel(x, cos_table, sin_table, wq, wkv_down, w_up, w_out, _want_perf=False):
    if "nc" not in _CACHE:
        _CACHE["nc"] = _build()
    nc = _CACHE["nc"]
    in_maps = _prep_inputs(x, cos_table, sin_table, wq, wkv_down, w_up, w_out)
    res = run_bass_kernel_spmd(nc, in_maps, core_ids=list(range(NCORE)),
                               trace=bool(_want_perf))
    out = np.zeros((B, L, E), np.float32)
    for core in range(NCORE):
        b = core // HPG
        out[b] += res.results[core]["outt"].astype(np.float32)
    if _want_perf:
        return out, res
    return out
